# revision 6
# baseline (speedup 1.0000x reference)
"""Trainium2 Bass kernel for nn_GaussianSplatter.

Two launches, data-parallel over batch (2 images per core x 8 cores):

launch 1 (reduce): logits are uploaded as fp8e4m3 in a host-pretransposed
  [h=112, k=100, b=2, w=112] layout (lbar is a mean of 4096 patches, so fp8
  quantization error averages out to ~6e-5).  The PE contracts the partition
  dim h -> r with a 0/1 selector S[h, r] = (h%7 == r), one 112-column matmul
  per (2-k chunk, k2, b), accumulating b pairs into PSUM.  Four chunks share
  each 112-partition PSUM fill via PE tile column offsets 0/32/64/96.  The
  DVE folds nw (fan 16) directly from PSUM into tiny per-fill partials that
  are DMA'd out; the host assembles the exact lbar from all cores.

host: Gaussian prototype math (covariance -> 5x5 kernels -> peak-norm ->
  pad -> bilinear translate -> opacity weight) in f64, exactly as the
  reference, producing Wm [49, 49].

launch 2 (apply): X = unfolded feat patches [49(p), 4096(b,c,nh,nw)] in
  bf16; 8 matmuls of 512 columns against stationary Wm (bf16) compute
  Y[ij, n] = sum_p Wm[p, ij] X[p, n].  Two PE tile groups (psum partition
  base 0 / 64) pack two 512-col outputs per PSUM bank so a single DVE/Act
  copy evacuates both; Y is written bf16 and folded on the host.
"""
import numpy as np
import ml_dtypes

import concourse.bass as bass
import concourse.mybir as mybir
from concourse.bass_utils import run_bass_kernel_spmd

N_CORES = 8
B, C_IN, H, W_ = 16, 64, 112, 112
K = 100
FEAT_C = 8
ROW = COL = 7
KSIZE = 5
P = 49
BPC = B // N_CORES              # images per core

NCHUNK = K // 2                 # 50 matmul chunks of 2 k's
NFILL = (NCHUNK + 3) // 4       # 13 psum fills of up to 4 chunks
DMA_K = [4, 8, 12, 16, 20, 20, 20]   # k's per input DMA chunk (sums to 100)

F8 = ml_dtypes.float8_e4m3
BF16 = ml_dtypes.bfloat16

_cache = {}


# ---------------------------------------------------------------- host math
def _translate_bilinear_np(img, sx, sy):
    Pn, Hh, Ww = img.shape
    ii = np.arange(Hh, dtype=img.dtype)[None, :, None] + sy[:, None, None]
    jj = np.arange(Ww, dtype=img.dtype)[None, None, :] + sx[:, None, None]
    ii = np.broadcast_to(ii, (Pn, Hh, Ww))
    jj = np.broadcast_to(jj, (Pn, Hh, Ww))
    i0 = np.floor(ii)
    j0 = np.floor(jj)
    wi = ii - i0
    wj = jj - j0
    pidx = np.arange(Pn)[:, None, None]

    def gather(iz, jz):
        valid = (iz >= 0) & (iz <= Hh - 1) & (jz >= 0) & (jz <= Ww - 1)
        v = img[pidx, np.clip(iz, 0, Hh - 1).astype(np.int32),
                np.clip(jz, 0, Ww - 1).astype(np.int32)]
        return np.where(valid, v, np.zeros((), img.dtype))

    v00 = gather(i0, j0)
    v01 = gather(i0, j0 + 1.0)
    v10 = gather(i0 + 1.0, j0)
    v11 = gather(i0 + 1.0, j0 + 1.0)
    return v00 * (1 - wi) * (1 - wj) + v01 * (1 - wi) * wj \
        + v10 * wi * (1 - wj) + v11 * wi * wj


def _compute_W(lbar, sigma_x, sigma_y, opacity, rho):
    lbar = lbar.astype(np.float64)
    wsx = lbar @ sigma_x.astype(np.float64)
    wsy = lbar @ sigma_y.astype(np.float64)
    wop = lbar @ opacity[:, 0].astype(np.float64)
    wrho = lbar @ rho[:, 0].astype(np.float64)
    a = wsx ** 2 + 1e-5
    d = wsy ** 2 + 1e-5
    b = wrho * wsx * wsy
    det = a * d - b * b
    ia, ib, idd = d / det, -b / det, a / det
    ax = np.linspace(-5.0, 5.0, KSIZE)
    xx = ax[:, None]
    yy = ax[None, :]
    z = -0.5 * (ia[:, None, None] * xx ** 2 + 2.0 * ib[:, None, None] * xx * yy
                + idd[:, None, None] * yy ** 2)
    kern = np.exp(z) / (2.0 * np.pi * np.sqrt(det)[:, None, None])
    kern = kern / kern.max(axis=(-2, -1), keepdims=True)
    ph, pw = ROW - KSIZE, COL - KSIZE
    kern = np.pad(kern, ((0, 0), (ph // 2, ph - ph // 2), (pw // 2, pw - pw // 2)))
    rr, cc = np.meshgrid(np.arange(ROW, dtype=np.float64),
                         np.arange(COL, dtype=np.float64), indexing='ij')
    tx = 1.0 - 2.0 * cc.reshape(-1) / ROW
    ty = 1.0 - 2.0 * rr.reshape(-1) / COL
    kT = _translate_bilinear_np(kern, tx * (COL - 1) / 2.0, ty * (ROW - 1) / 2.0)
    return (wop[:, None] * kT.reshape(P, P)).astype(np.float32)


# ------------------------------------------------------------- launch 1 IR
def _build_reduce_nc():
    nc = bass.Bass()
    f32 = mybir.dt.float32
    f8 = mybir.dt.float8e4
    lgT = nc.declare_dram_parameter("lgT", [112, K, BPC, 112], f8, isOutput=False)
    S = nc.declare_dram_parameter("S", [112, 7], f8, isOutput=False)
    r2 = nc.declare_dram_parameter("r2", [112, NFILL, 2, 7], f32, isOutput=True)

    # chunk c (k = 2c, 2c+1) is covered by input-DMA chunk index:
    kcum = np.concatenate([[0], np.cumsum(DMA_K)])

    def dma_chunk_of(c):
        hi = 2 * c + 2
        for j in range(len(DMA_K)):
            if kcum[j + 1] >= hi:
                return j
        raise AssertionError

    with bass.ExitStack() as ctx:
        LG = ctx.enter_context(nc.sbuf_tensor([112, K, BPC, 112], f8))
        SW = ctx.enter_context(nc.sbuf_tensor([112, 7], f8))
        R2 = ctx.enter_context(nc.sbuf_tensor([112, NFILL, 2, 7], f32))
        PS = [ctx.enter_context(nc.psum_tensor(f"ps{i}", [112, 224], f32))
              for i in range(4)]
        ssem = ctx.enter_context(nc.semaphore("ss"))
        csems = [ctx.enter_context(nc.semaphore(f"c{j}"))
                 for j in range(len(DMA_K))]
        fsem = ctx.enter_context(nc.semaphore("f"))
        rsem = ctx.enter_context(nc.semaphore("r"))
        osem = ctx.enter_context(nc.semaphore("o"))
        block = ctx.enter_context(nc.Block())

        @block.sync
        def _(sync):
            sync.dma_start(SW[:], S[:]).then_inc(ssem, 16)
            for j, nk in enumerate(DMA_K):
                sl = slice(kcum[j], kcum[j + 1])
                sync.dma_start(LG[:, sl], lgT[:, sl]).then_inc(csems[j], 16)
            sync.wait_ge(osem, 32)

        @block.scalar
        def _(scalar):
            scalar.wait_ge(rsem, 10)
            scalar.dma_start(r2[:, 0:10], R2[:, 0:10]).then_inc(osem, 16)
            scalar.wait_ge(rsem, NFILL)
            scalar.dma_start(r2[:, 10:NFILL], R2[:, 10:NFILL]).then_inc(osem, 16)

        @block.tensor
        def _(tensor):
            tensor.wait_ge(ssem, 16)
            cur_chunk = -1
            for c in range(NCHUNK):
                f, q = c // 4, c % 4
                j = dma_chunk_of(c)
                if j > cur_chunk:
                    tensor.wait_ge(csems[j], 16)
                    cur_chunk = j
                if q == 0 and f >= 4:
                    tensor.wait_ge(rsem, f - 3)
                for k2 in range(2):
                    for bb in range(BPC):
                        ins = nc.tensor.matmul(
                            PS[f % 4][32 * q:32 * q + 7, 112 * k2:112 * k2 + 112],
                            SW[:],
                            LG[:, 2 * c + k2, bb, :],
                            start=(bb == 0), stop=(bb == BPC - 1),
                            tile_position=(0, 32 * q))
                if q == 3 or c == NCHUNK - 1:
                    ins.then_inc(fsem, 1)

        @block.vector
        def _(vector):
            for f in range(NFILL):
                vector.wait_ge(fsem, f + 1)
                nc.vector.reduce_sum(
                    R2[:, f],
                    PS[f % 4][:].rearrange("p (k nw cc) -> p k cc nw", k=2, cc=7),
                    axis=mybir.AxisListType.X).then_inc(rsem, 1)
    return nc


# ------------------------------------------------------------- launch 2 IR
def _build_apply_nc():
    nc = bass.Bass()
    f32 = mybir.dt.float32
    bf = mybir.dt.bfloat16
    Xp = nc.declare_dram_parameter("X", [49, 4096], bf, isOutput=False)
    Wp = nc.declare_dram_parameter("Wm", [49, 49], bf, isOutput=False)
    yp = nc.declare_dram_parameter("y", [113, 4, 512], bf, isOutput=True)

    with bass.ExitStack() as ctx:
        XS = ctx.enter_context(nc.sbuf_tensor([49, 4096], bf))
        WS = ctx.enter_context(nc.sbuf_tensor([49, 49], bf))
        OUT = ctx.enter_context(nc.sbuf_tensor([128, 4, 512], bf))
        PS = [ctx.enter_context(nc.psum_tensor(f"ps{i}", [128, 512], f32))
              for i in range(4)]
        wsem = ctx.enter_context(nc.semaphore("w"))
        x1sem = ctx.enter_context(nc.semaphore("x1"))
        x2sem = ctx.enter_context(nc.semaphore("x2"))
        msem = ctx.enter_context(nc.semaphore("m"))
        e1sem = ctx.enter_context(nc.semaphore("e1"))
        e2sem = ctx.enter_context(nc.semaphore("e2"))
        osem = ctx.enter_context(nc.semaphore("o"))
        block = ctx.enter_context(nc.Block())

        @block.sync
        def _(sync):
            sync.dma_start(WS[:], Wp[:]).then_inc(wsem, 16)
            sync.dma_start(XS[:, 0:2048], Xp[:, 0:2048]).then_inc(x1sem, 16)
            sync.dma_start(XS[:, 2048:4096], Xp[:, 2048:4096]).then_inc(x2sem, 16)
            sync.wait_ge(e1sem, 2)
            sync.dma_start(yp[:, 0:2], OUT[0:113, 0:2]).then_inc(osem, 16)
            sync.wait_ge(e2sem, 2)
            sync.dma_start(yp[:, 2:4], OUT[0:113, 2:4]).then_inc(osem, 16)
            sync.wait_ge(osem, 32)

        @block.tensor
        def _(tensor):
            tensor.wait_ge(wsem, 16)
            tensor.wait_ge(x1sem, 16)
            for i in range(4):          # group A: image 0, psum partitions 0-48
                nc.tensor.matmul(
                    PS[i][0:49, :], WS[:], XS[:, 512 * i:512 * i + 512],
                    start=True, stop=True, tile_position=(0, 0))
            tensor.wait_ge(x2sem, 16)
            for i in range(4):          # group B: image 1, psum partitions 64-112
                nc.tensor.matmul(
                    PS[i][64:113, :], WS[:], XS[:, 2048 + 512 * i:2048 + 512 * i + 512],
                    start=True, stop=True,
                    tile_position=(0, 64)).then_inc(msem, 1)

        @block.vector
        def _(vector):
            vector.wait_ge(msem, 1)
            nc.vector.tensor_copy(OUT[0:113, 0], PS[0][0:113, :]).then_inc(e1sem, 1)
            vector.wait_ge(msem, 3)
            nc.vector.tensor_copy(OUT[0:113, 2], PS[2][0:113, :]).then_inc(e2sem, 1)

        @block.scalar
        def _(scalar):
            scalar.wait_ge(msem, 2)
            nc.scalar.copy(OUT[0:113, 1], PS[1][0:113, :]).then_inc(e1sem, 1)
            scalar.wait_ge(msem, 4)
            nc.scalar.copy(OUT[0:113, 3], PS[2 + 1][0:113, :]).then_inc(e2sem, 1)
    return nc


# ------------------------------------------------------------------ driver
def kernel(inp, logits, sigma_x, sigma_y, opacity, rho, scale):
    inp = np.asarray(inp)
    logits = np.asarray(logits, dtype=np.float32)
    feat = np.asarray(inp[:, :FEAT_C], dtype=np.float32)

    if "reduce" not in _cache:
        _cache["reduce"] = _build_reduce_nc()
    if "apply" not in _cache:
        _cache["apply"] = _build_apply_nc()

    core_ids = list(range(N_CORES))

    # ---- launch 1: logits reduction -------------------------------------
    Sarr = np.zeros((112, 7), np.float32)
    Sarr[np.arange(112), np.arange(112) % 7] = 1.0
    Sarr = Sarr.astype(F8)
    lg8 = logits.astype(F8)             # fp8 quantization (averages out)
    in_maps1 = []
    for i in core_ids:
        lgT = np.ascontiguousarray(
            lg8[2 * i:2 * i + 2].transpose(2, 1, 0, 3))   # [h, k, b, w]
        in_maps1.append({"lgT": lgT, "S": Sarr})
    res1 = run_bass_kernel_spmd(_cache["reduce"], in_maps1, core_ids)

    # assemble exact lbar [49, 100] from per-fill partials
    r2sum = np.zeros((112, NFILL, 2, 7), np.float64)
    for i in core_ids:
        r2sum += res1.results[i]["r2"].astype(np.float64)
    ksum = np.zeros((K, 7, 7), np.float64)       # [k, r, cc]
    for c in range(NCHUNK):
        f, q = c // 4, c % 4
        blk = r2sum[32 * q:32 * q + 7, f]        # [r, k2, cc]
        ksum[2 * c] = blk[:, 0, :]
        ksum[2 * c + 1] = blk[:, 1, :]
    lbar = ksum.transpose(1, 2, 0).reshape(P, K) / (B * 16 * 16)

    # ---- host: Gaussian prototype math ----------------------------------
    Wm = _compute_W(lbar, np.asarray(sigma_x), np.asarray(sigma_y),
                    np.asarray(opacity), np.asarray(rho))

    # ---- launch 2: apply ------------------------------------------------
    Wq = Wm.astype(BF16)
    in_maps2 = []
    for i in core_ids:
        v = feat[2 * i:2 * i + 2].reshape(2, FEAT_C, 16, 7, 16, 7)
        X = np.ascontiguousarray(
            v.transpose(3, 5, 0, 1, 2, 4).reshape(49, 4096)).astype(BF16)
        in_maps2.append({"X": X, "Wm": Wq})
    res2 = run_bass_kernel_spmd(_cache["apply"], in_maps2, core_ids)

    out = np.empty((B, FEAT_C, H, W_), np.float32)
    for i in core_ids:
        y = np.asarray(res2.results[i]["y"]).astype(np.float32).reshape(113, 2048)
        for g, row0 in ((0, 0), (1, 64)):
            img = y[row0:row0 + 49].reshape(7, 7, FEAT_C, 16, 16)
            out[2 * i + g] = img.transpose(2, 3, 0, 4, 1).reshape(FEAT_C, H, W_)
    return out
